# revision 6
# baseline (speedup 1.0000x reference)
"""Trainium2 Bass kernel for nn_AttentionNet (gnn_message_passing).

Full inputs -> full output. Internally: pure data parallel over 8 NeuronCores
(batch N=128 -> 16 samples/core). All compute fused on-chip:

  h = relu(fc(x))                        (per-pair (2 samples) SBUF tiles)
  3x attention-vector stages, each:
      lin[i,(j,k)] = W01*X[n,i,k] + W00*X[n,j,k]     (ONE K=66 matmul/pair:
        64 delta-rows select X^T columns, 2 indicator rows carry flattened X)
      S = sigmoid(lin + b)               (ScalarE, straight from PSUM)
      T[i,j] = sum_k S                   (VectorE reduce)
      m = (sum_j T)/(64*(max_j T + 64*eps)); z = relu(sharp*m + th)
      softmax over i (TensorE transpose + sigmoid-ratio exp trick)
      vec = softmax/(max softmax + eps) = e/(1 + eps*sum e)   [exact]

Scalars (weights/biases/sharps) are folded in as immediates at build time.
"""

import sys

import numpy as np

for _p in ("/opt/trn_rl_repo",):
    if _p not in sys.path:
        sys.path.append(_p)

import concourse.bacc as bacc
import concourse.bass as bass
import concourse.mybir as mybir
import concourse.tile as tile
from concourse.bass_utils import run_bass_kernel_spmd
from concourse.masks import make_identity

F32 = mybir.dt.float32
AF = mybir.ActivationFunctionType
ALU = mybir.AluOpType
AX = mybir.AxisListType

EPS = 1e-6
N_CORES = 8
N_FULL = 128
NLOC = N_FULL // N_CORES  # 16 samples per core
NPAIR = NLOC // 2  # 8 partition-pairs per core
A = 64  # a1 == a2 == 64
AA = A * A  # 4096
NQ = 4  # quarters per pair (j in groups of 16)
JQ = A // NQ  # 16 j per quarter
NCH = 2  # matmul chunks per quarter (N=512 each)


def _build(p, debug=False):
    """p: dict of python-float params. Returns compiled Bacc."""
    nc = bacc.Bacc("TRN2", target_bir_lowering=False, debug=False,
                   num_devices=N_CORES)
    x_ext = nc.dram_tensor("x", [NLOC, 2, A, A], F32, kind="ExternalInput")
    out_ext = nc.dram_tensor("out", [NLOC, A], F32, kind="ExternalOutput")
    dbg = {}
    if debug:
        for name, shape in [("dbg_h0", [NPAIR, 128, A]), ("dbg_c1", [NPAIR, 128, A]),
                            ("dbg_z1", [128, NPAIR]), ("dbg_vec1", [NPAIR, 128]),
                            ("dbg_x3", [NPAIR, 128, A]), ("dbg_vec2", [NPAIR, 128])]:
            dbg[name] = nc.dram_tensor(name, shape, F32, kind="ExternalOutput")

    with tile.TileContext(nc) as tc:
        _body(tc, x_ext, out_ext, p, dbg)
    nc.compile()
    return nc


def _body(tc, x_ext, out_ext, p, dbg):
    nc = tc.nc

    # ---- persistent SBUF tensors -------------------------------------
    ident = nc.alloc_sbuf_tensor("ident", [128, 128], F32).ap()
    ind2 = nc.alloc_sbuf_tensor("ind2", [2, 128], F32).ap()
    # rhs mega tensors: [0:64] delta const, [64:66] flattened V rows (per pair)
    megaJK = [nc.alloc_sbuf_tensor(f"megaJK{i}", [66, AA], F32).ap() for i in range(2)]
    megaKJ = [nc.alloc_sbuf_tensor(f"megaKJ{i}", [66, AA], F32).ap() for i in range(2)]

    def ptile(name):
        return [nc.alloc_sbuf_tensor(f"{name}{q}", [128, A], F32).ap()
                for q in range(NPAIR)]

    x0t, x1t = ptile("x0"), ptile("x1")
    h0, h1 = ptile("h0"), ptile("h1")
    h0s, h1s = ptile("h0s"), ptile("h1s")
    x3 = ptile("x3")

    # ---- consts ------------------------------------------------------
    make_identity(nc, ident)

    def make_indicator(t, val):
        # t[r, m] = val if m // 64 == r else 0   (r in {0,1}, m in [0,128))
        nc.gpsimd.memset(t, float(val))
        # keep where (m - 64*r) >= 0, else 0
        nc.gpsimd.affine_select(out=t, in_=t, compare_op=ALU.is_ge, fill=0.0,
                                base=0, pattern=[[1, 128]], channel_multiplier=-64)
        # keep where (63 - m + 64*r) >= 0, else 0
        nc.gpsimd.affine_select(out=t, in_=t, compare_op=ALU.is_ge, fill=0.0,
                                base=63, pattern=[[-1, 128]], channel_multiplier=64)

    make_indicator(ind2, 1.0)
    indT = {}
    for nm, w00 in (("i1", p["w00_1"]), ("i2", p["w00_2"]), ("i3", p["w00_3"])):
        t = nc.alloc_sbuf_tensor(f"indT_{nm}", [2, 128], F32).ap()
        make_indicator(t, w00)
        indT[nm] = t
    for m in megaJK:  # delta[kk, (j,k)] = (k == kk)
        v = m[0:64, :].rearrange("p (j k) -> p j k", k=A)
        nc.gpsimd.memset(v, 0.0)
        nc.gpsimd.affine_select(out=v, in_=v, compare_op=ALU.not_equal, fill=1.0,
                                base=0, pattern=[[0, A], [1, A]], channel_multiplier=-1)
    for m in megaKJ:  # delta[kk, (k,j)] = W01_2 * (k == kk)
        v = m[0:64, :].rearrange("p (k j) -> p k j", j=A)
        nc.gpsimd.memset(v, 0.0)
        nc.gpsimd.affine_select(out=v, in_=v, compare_op=ALU.not_equal,
                                fill=float(p["w01_2"]), base=0,
                                pattern=[[1, A], [0, A]], channel_multiplier=-1)

    # ---- pools -------------------------------------------------------
    import contextlib
    ctx = contextlib.ExitStack()
    lhsT_pool = ctx.enter_context(tc.tile_pool(name="lhsT", bufs=4))
    s_pool = ctx.enter_context(tc.tile_pool(name="sig", bufs=3))
    c_pool = ctx.enter_context(tc.tile_pool(name="cml", bufs=3))
    st_pool = ctx.enter_context(tc.tile_pool(name="stat", bufs=3))
    post_pool = ctx.enter_context(tc.tile_pool(name="post", bufs=2))
    lin_pool = ctx.enter_context(tc.tile_pool(name="linp", bufs=3, space="PSUM"))
    misc_pool = ctx.enter_context(tc.tile_pool(name="miscp", bufs=2, space="PSUM"))

    # ---- load x + fc -------------------------------------------------
    for q in range(NPAIR):
        for nl in range(2):
            nc.sync.dma_start(out=x0t[q][64 * nl:64 * nl + 64, :],
                              in_=x_ext[2 * q + nl, 0, :, :])
            nc.sync.dma_start(out=x1t[q][64 * nl:64 * nl + 64, :],
                              in_=x_ext[2 * q + nl, 1, :, :])
        for ch, ht in ((0, h0), (1, h1)):
            s1 = st_pool.tile([128, A], F32, tag="fc_s1")
            nc.vector.tensor_scalar(out=s1, in0=x1t[q], scalar1=float(p["fc"][ch][1]),
                                    scalar2=float(p["fcb"][ch]), op0=ALU.mult,
                                    op1=ALU.add)
            nc.vector.scalar_tensor_tensor(out=ht[q], in0=x0t[q],
                                           scalar=float(p["fc"][ch][0]), in1=s1,
                                           op0=ALU.mult, op1=ALU.add)
            nc.vector.tensor_scalar_max(out=ht[q], in0=ht[q], scalar1=0.0)
    if dbg:
        for q in range(NPAIR):
            nc.sync.dma_start(out=dbg["dbg_h0"][q], in_=h0[q])

    # ---- attention stages --------------------------------------------
    def att(xt, megas, jk, indt, w01, b, sharp, th, last, name):
        """Returns vec_t (8,128) tile [(pair),(nl,i)] and Z for dbg."""
        zcol = post_pool.tile([128, NPAIR], F32, tag="Z")
        bias_t = post_pool.tile([128, 1], F32, tag="bias")
        nc.gpsimd.memset(bias_t, float(b))
        z8 = post_pool.tile([NPAIR, 1], F32, tag="z8")
        nc.gpsimd.memset(z8, 0.0)
        for q in range(NPAIR):
            mega = megas[q % 2]
            lhsT = lhsT_pool.tile([66, 128], F32, tag="lhsT")
            if jk:
                tp = misc_pool.tile([64, 128], F32, tag="m")
                nc.tensor.transpose(tp, xt[q], ident)
                nc.vector.tensor_scalar_mul(out=lhsT[0:64, :], in0=tp,
                                            scalar1=float(w01))
            else:
                for nl in range(2):
                    nc.sync.dma_start(out=lhsT[0:64, 64 * nl:64 * nl + 64],
                                      in_=xt[q][64 * nl:64 * nl + 64, :])
            nc.sync.dma_start(out=lhsT[64:66, :], in_=indt)
            for nl in range(2):
                nc.sync.dma_start(out=mega[64 + nl:65 + nl, :],
                                  in_=xt[q][64 * nl:64 * nl + 64, :])

            cml = c_pool.tile([128, A], F32, tag="c")
            for qq in range(NQ):
                lin = lin_pool.tile([128, NCH * 512], F32, tag="lin")
                for h in range(NCH):
                    if jk:  # free = (j major, k minor); chunk = 8 j x 64 k
                        j0 = JQ * qq + 8 * h
                        rhs = mega[0:66, j0 * A:(j0 + 8) * A]
                    else:  # free = (k major, j minor); chunk = 32 k x 16 j
                        mv = mega[0:66, :].rearrange("p (k j) -> p k j", j=A)
                        rhs = mv[:, 32 * h:32 * h + 32, JQ * qq:JQ * qq + JQ]
                    nc.tensor.matmul(lin[:, 512 * h:512 * (h + 1)], lhsT[0:66, :],
                                     rhs, start=True, stop=True)
                sig = s_pool.tile([128, NCH * 512], F32, tag="S")
                nc.scalar.activation(sig, lin, AF.Sigmoid, bias=bias_t, scale=1.0)
                if jk:
                    sv = sig.rearrange("p (j k) -> p j k", k=A)
                else:
                    sv = sig.rearrange("p (k j) -> p j k", j=JQ)
                nc.vector.reduce_sum(out=cml[:, JQ * qq:JQ * (qq + 1)], in_=sv,
                                     axis=AX.X)
            if dbg and name == "a1":
                nc.sync.dma_start(out=dbg["dbg_c1"][q], in_=cml)
            maxT = st_pool.tile([128, 1], F32, tag="maxT")
            sumT = st_pool.tile([128, 1], F32, tag="sumT")
            nc.vector.reduce_max(out=maxT, in_=cml, axis=AX.X)
            nc.vector.reduce_sum(out=sumT, in_=cml, axis=AX.X)
            rin = st_pool.tile([128, 1], F32, tag="rin")
            nc.vector.tensor_scalar_add(out=rin, in0=maxT, scalar1=float(A * EPS))
            rec = st_pool.tile([128, 1], F32, tag="rec")
            nc.vector.reciprocal(rec, rin)
            t1 = st_pool.tile([128, 1], F32, tag="t1")
            nc.vector.tensor_mul(out=t1, in0=sumT, in1=rec)
            nc.vector.tensor_scalar(out=zcol[:, q:q + 1], in0=t1,
                                    scalar1=float(sharp) / A, scalar2=float(th),
                                    op0=ALU.mult, op1=ALU.add)
            nc.vector.tensor_scalar_max(out=zcol[:, q:q + 1], in0=zcol[:, q:q + 1],
                                        scalar1=0.0)
        # softmax over i (= 64-blocks of the partition dim)
        ztp = misc_pool.tile([NPAIR, 128], F32, tag="m")
        nc.tensor.transpose(ztp, zcol, ident)
        zt = post_pool.tile([NPAIR, 128], F32, tag="zt")
        nc.vector.tensor_copy(zt, ztp)
        vec_t = post_pool.tile([NPAIR, 128], F32, tag="vt")
        for nl in range(2):
            blk = zt[:, 64 * nl:64 * nl + 64]
            mx = st_pool.tile([NPAIR, 1], F32, tag="mx")
            nc.vector.reduce_max(out=mx, in_=blk, axis=AX.X)
            nmx = st_pool.tile([NPAIR, 1], F32, tag="nmx")
            nc.vector.tensor_scalar_mul(out=nmx, in0=mx, scalar1=-1.0)
            u = post_pool.tile([NPAIR, 64], F32, tag="u")
            nc.vector.tensor_scalar_add(out=u, in0=blk, scalar1=nmx)
            sp = post_pool.tile([NPAIR, 64], F32, tag="sp")
            nc.scalar.activation(sp, u, AF.Sigmoid, bias=z8)
            sn = post_pool.tile([NPAIR, 64], F32, tag="sn")
            nc.scalar.activation(sn, u, AF.Sigmoid, bias=z8, scale=-1.0)
            rn = post_pool.tile([NPAIR, 64], F32, tag="rn")
            nc.vector.reciprocal(rn, sn)
            e = post_pool.tile([NPAIR, 64], F32, tag="e")
            nc.vector.tensor_mul(out=e, in0=sp, in1=rn)
            s = st_pool.tile([NPAIR, 1], F32, tag="s")
            nc.vector.reduce_sum(out=s, in_=e, axis=AX.X)
            fr = st_pool.tile([NPAIR, 1], F32, tag="fr")
            if last:
                nc.vector.reciprocal(fr, s)
            else:
                sf = st_pool.tile([NPAIR, 1], F32, tag="sf")
                nc.vector.tensor_scalar(out=sf, in0=s, scalar1=EPS, scalar2=1.0,
                                        op0=ALU.mult, op1=ALU.add)
                nc.vector.reciprocal(fr, sf)
            nc.vector.tensor_scalar_mul(out=vec_t[:, 64 * nl:64 * nl + 64], in0=e,
                                        scalar1=fr)
        if dbg and name == "a1":
            nc.sync.dma_start(out=dbg["dbg_z1"], in_=zcol)
            nc.sync.dma_start(out=dbg["dbg_vec1"], in_=vec_t)
        if dbg and name == "a2":
            nc.sync.dma_start(out=dbg["dbg_vec2"], in_=vec_t)
        return vec_t

    # att1 on h0 (JK layout)
    vec1_t = att(h0, megaJK, True, indT["i1"], p["w01_1"], p["b_1"],
                 p["sharp_1"], p["th_1"], False, "a1")
    vcp = misc_pool.tile([128, NPAIR], F32, tag="m")
    nc.tensor.transpose(vcp, vec1_t, ident[0:NPAIR, 0:NPAIR])
    vcols = post_pool.tile([128, NPAIR], F32, tag="vcols")
    nc.vector.tensor_copy(vcols, vcp)
    for q in range(NPAIR):
        nc.vector.tensor_scalar_mul(out=h0s[q], in0=h0[q], scalar1=vcols[:, q:q + 1])
        nc.vector.tensor_scalar_mul(out=h1s[q], in0=h1[q], scalar1=vcols[:, q:q + 1])

    # att2 on transpose(h1s) (KJ layout; w01 folded into deltaKJ fill)
    vec2_t = att(h1s, megaKJ, False, indT["i2"], p["w01_2"], p["b_2"],
                 p["sharp_2"], p["th_2"], False, "a2")
    # vnq[nl, pair*64+q] = vec2[2*pair+nl, q]  (base-partition-0 slices for matmul)
    vnq = post_pool.tile([2, NPAIR * 64], F32, tag="vnq")
    for nl in range(2):
        nc.sync.dma_start(out=vnq[nl:nl + 1, :], in_=vec2_t[:, 64 * nl:64 * nl + 64])
    for q in range(NPAIR):
        vb = misc_pool.tile([128, 64], F32, tag="m")
        nc.tensor.matmul(vb, ind2, vnq[0:2, 64 * q:64 * q + 64], start=True,
                         stop=True)
        nc.vector.tensor_mul(out=x3[q], in0=h0s[q], in1=vb)
    if dbg:
        for q in range(NPAIR):
            nc.sync.dma_start(out=dbg["dbg_x3"][q], in_=x3[q])

    # att3 on x3 (JK layout), is_last
    vec3_t = att(x3, megaJK, True, indT["i3"], p["w01_3"], p["b_3"],
                 p["sharp_3"], p["th_3"], True, "a3")
    out_r = out_ext.rearrange("(q t) a -> q t a", t=2)
    for nl in range(2):
        nc.sync.dma_start(out=out_r[:, nl, :], in_=vec3_t[:, 64 * nl:64 * nl + 64])
    ctx.close()


_CACHE = {}


def _params(fc_W, fc_b, p1_W, p1_b, p1_sharp, p1_th, p2_W, p2_b, p2_sharp, p2_th,
            out_W, out_b, out_sharp, out_th):
    return {
        "fc": [[float(fc_W[0, 0]), float(fc_W[0, 1])],
               [float(fc_W[1, 0]), float(fc_W[1, 1])]],
        "fcb": [float(fc_b[0]), float(fc_b[1])],
        "w00_1": float(p1_W[0, 0]), "w01_1": float(p1_W[0, 1]),
        "b_1": float(p1_b[0]), "sharp_1": float(p1_sharp[0]), "th_1": float(p1_th[0]),
        "w00_2": float(p2_W[0, 0]), "w01_2": float(p2_W[0, 1]),
        "b_2": float(p2_b[0]), "sharp_2": float(p2_sharp[0]), "th_2": float(p2_th[0]),
        "w00_3": float(out_W[0, 0]), "w01_3": float(out_W[0, 1]),
        "b_3": float(out_b[0]), "sharp_3": float(out_sharp[0]),
        "th_3": float(out_th[0]),
    }


def kernel(x, fc_W, fc_b, p1_W, p1_b, p1_sharp, p1_th, p2_W, p2_b, p2_sharp, p2_th,
           out_W, out_b, out_sharp, out_th, _debug=False, _trace=False):
    x = np.ascontiguousarray(np.asarray(x, dtype=np.float32))
    assert x.shape == (N_FULL, 2, A, A), x.shape
    p = _params(fc_W, fc_b, p1_W, p1_b, p1_sharp, p1_th, p2_W, p2_b, p2_sharp,
                p2_th, out_W, out_b, out_sharp, out_th)
    key = (tuple(sorted((k, str(v)) for k, v in p.items())), _debug)
    if key not in _CACHE:
        _CACHE[key] = _build(p, debug=_debug)
    nc = _CACHE[key]
    in_maps = [{"x": x[i * NLOC:(i + 1) * NLOC]} for i in range(N_CORES)]
    r = run_bass_kernel_spmd(nc, in_maps, list(range(N_CORES)), trace=_trace)
    out = np.concatenate([r.results[i]["out"] for i in range(N_CORES)], axis=0)
    if _debug or _trace:
        return out, r
    return out


# revision 7
# speedup vs baseline: 1.4645x; 1.4645x over previous
"""Trainium2 Bass kernel for nn_AttentionNet (gnn_message_passing).

Full inputs -> full output. Internally: pure data parallel over 8 NeuronCores
(batch N=128 -> 16 samples/core). All compute fused on-chip:

  h = relu(fc(x))                        (per-pair (2 samples) SBUF tiles)
  3x attention-vector stages, each:
      lin[i,(j,k)] = W01*X[n,i,k] + W00*X[n,j,k]     (ONE K=66 bf16 matmul/pair:
        64 delta-rows select X^T columns, 2 indicator rows carry flattened X)
      S = sigmoid(lin + b)               (ScalarE, straight from PSUM, bf16 out)
      T[i,j] = sum_k S                   (GPSIMD round-1 add + VectorE reduce)
      m = (sum_j T)/(64*(max_j T + 64*eps)); z = relu(sharp*m + th)
      softmax over i (TensorE transpose + sigmoid-ratio exp trick)
      vec = softmax/(max softmax + eps) = e/(1 + eps*sum e)   [exact]

Elementwise h-chain stays f32; only matmul operands are bf16.
Scalars (weights/biases/sharps) are folded in as immediates at build time.
"""

import sys

import numpy as np

for _p in ("/opt/trn_rl_repo",):
    if _p not in sys.path:
        sys.path.append(_p)

import concourse.bacc as bacc
import concourse.bass as bass
import concourse.mybir as mybir
import concourse.tile as tile
from concourse.bass_utils import run_bass_kernel_spmd
from concourse.masks import make_identity

F32 = mybir.dt.float32
BF16 = mybir.dt.bfloat16
AF = mybir.ActivationFunctionType
ALU = mybir.AluOpType
AX = mybir.AxisListType

EPS = 1e-6
N_CORES = 8
N_FULL = 128
NLOC = N_FULL // N_CORES  # 16 samples per core
NPAIR = NLOC // 2  # 8 partition-pairs per core
A = 64  # a1 == a2 == 64
AA = A * A  # 4096
HJ = 32  # j per half
NCH = 4  # matmul chunks per half (N=512 each)


def _build(p, debug=False):
    """p: dict of python-float params. Returns compiled Bacc."""
    nc = bacc.Bacc("TRN2", target_bir_lowering=False, debug=False,
                   num_devices=N_CORES)
    x_ext = nc.dram_tensor("x", [NLOC, 2, A, A], F32, kind="ExternalInput")
    out_ext = nc.dram_tensor("out", [NLOC, A], F32, kind="ExternalOutput")
    dbg = {}
    if debug:
        for name, shape in [("dbg_h0", [NPAIR, 128, A]), ("dbg_c1", [NPAIR, 128, A]),
                            ("dbg_z1", [128, NPAIR]), ("dbg_vec1", [NPAIR, 128]),
                            ("dbg_x3", [NPAIR, 128, A]), ("dbg_vec2", [NPAIR, 128])]:
            dbg[name] = nc.dram_tensor(name, shape, F32, kind="ExternalOutput")

    with tile.TileContext(nc) as tc:
        _body(tc, x_ext, out_ext, p, dbg)
    nc.compile()
    return nc


def _body(tc, x_ext, out_ext, p, dbg):
    nc = tc.nc

    # ---- persistent SBUF tensors -------------------------------------
    ident = nc.alloc_sbuf_tensor("ident", [128, 128], F32).ap()
    identb = nc.alloc_sbuf_tensor("identb", [128, 128], BF16).ap()
    ind2 = nc.alloc_sbuf_tensor("ind2", [2, 128], BF16).ap()
    # rhs mega tensors: [0:64] delta const, [64:66] flattened V rows (per pair)
    megaJK = [nc.alloc_sbuf_tensor(f"megaJK{i}", [66, AA], BF16).ap()
              for i in range(2)]
    megaKJ = [nc.alloc_sbuf_tensor(f"megaKJ{i}", [66, AA], BF16).ap()
              for i in range(2)]

    def ptile(name, dt=F32):
        return [nc.alloc_sbuf_tensor(f"{name}{q}", [128, A], dt).ap()
                for q in range(NPAIR)]

    x0t, x1t = ptile("x0"), ptile("x1")
    h0, h1 = ptile("h0"), ptile("h1")
    h0s, h1s = ptile("h0s"), ptile("h1s")
    x3 = ptile("x3")

    # ---- consts ------------------------------------------------------
    make_identity(nc, ident)
    make_identity(nc, identb)

    def make_indicator(t, val):
        # t[r, m] = val if m // 64 == r else 0   (r in {0,1}, m in [0,128))
        nc.gpsimd.memset(t, float(val))
        nc.gpsimd.affine_select(out=t, in_=t, compare_op=ALU.is_ge, fill=0.0,
                                base=0, pattern=[[1, 128]], channel_multiplier=-64)
        nc.gpsimd.affine_select(out=t, in_=t, compare_op=ALU.is_ge, fill=0.0,
                                base=63, pattern=[[-1, 128]], channel_multiplier=64)

    make_indicator(ind2, 1.0)
    indT = {}
    for nm, w00 in (("i1", p["w00_1"]), ("i2", p["w00_2"]), ("i3", p["w00_3"])):
        t = nc.alloc_sbuf_tensor(f"indT_{nm}", [2, 128], BF16).ap()
        make_indicator(t, w00)
        indT[nm] = t
    for m in megaJK:  # delta[kk, (j,k)] = (k == kk)
        v = m[0:64, :].rearrange("p (j k) -> p j k", k=A)
        nc.gpsimd.memset(v, 0.0)
        nc.gpsimd.affine_select(out=v, in_=v, compare_op=ALU.not_equal, fill=1.0,
                                base=0, pattern=[[0, A], [1, A]], channel_multiplier=-1)
    for m in megaKJ:  # delta[kk, (k,j)] = (k == kk)
        v = m[0:64, :].rearrange("p (k j) -> p k j", j=A)
        nc.gpsimd.memset(v, 0.0)
        nc.gpsimd.affine_select(out=v, in_=v, compare_op=ALU.not_equal, fill=1.0,
                                base=0, pattern=[[1, A], [0, A]], channel_multiplier=-1)

    # ---- pools -------------------------------------------------------
    import contextlib
    ctx = contextlib.ExitStack()
    xb_pool = ctx.enter_context(tc.tile_pool(name="xb", bufs=4))
    lhsT_pool = ctx.enter_context(tc.tile_pool(name="lhsT", bufs=4))
    s_pool = ctx.enter_context(tc.tile_pool(name="sig", bufs=3))
    t1_pool = ctx.enter_context(tc.tile_pool(name="t1", bufs=3))
    c_pool = ctx.enter_context(tc.tile_pool(name="cml", bufs=3))
    st_pool = ctx.enter_context(tc.tile_pool(name="stat", bufs=3))
    post_pool = ctx.enter_context(tc.tile_pool(name="post", bufs=2))
    psum_pool = ctx.enter_context(tc.tile_pool(name="psum", bufs=2, space="PSUM"))

    # ---- load x + fc -------------------------------------------------
    for q in range(NPAIR):
        for nl in range(2):
            nc.sync.dma_start(out=x0t[q][64 * nl:64 * nl + 64, :],
                              in_=x_ext[2 * q + nl, 0, :, :])
            nc.sync.dma_start(out=x1t[q][64 * nl:64 * nl + 64, :],
                              in_=x_ext[2 * q + nl, 1, :, :])
        for ch, ht in ((0, h0), (1, h1)):
            s1 = st_pool.tile([128, A], F32, tag="fc_s1")
            nc.vector.tensor_scalar(out=s1, in0=x1t[q], scalar1=float(p["fc"][ch][1]),
                                    scalar2=float(p["fcb"][ch]), op0=ALU.mult,
                                    op1=ALU.add)
            nc.vector.scalar_tensor_tensor(out=ht[q], in0=x0t[q],
                                           scalar=float(p["fc"][ch][0]), in1=s1,
                                           op0=ALU.mult, op1=ALU.add)
            nc.vector.tensor_scalar_max(out=ht[q], in0=ht[q], scalar1=0.0)
    if dbg:
        for q in range(NPAIR):
            nc.sync.dma_start(out=dbg["dbg_h0"][q], in_=h0[q])

    # ---- attention stages --------------------------------------------
    def att(xt, megas, jk, indt, w01, b, sharp, th, last, name):
        """Returns vec_t (8,128) tile [(pair),(nl,i)]."""
        zcol = post_pool.tile([128, NPAIR], F32, tag="Z")
        bias_t = post_pool.tile([128, 1], F32, tag="bias")
        nc.gpsimd.memset(bias_t, float(b))
        z8 = post_pool.tile([NPAIR, 1], F32, tag="z8")
        nc.gpsimd.memset(z8, 0.0)
        for q in range(NPAIR):
            mega = megas[q % 2]
            xb = xb_pool.tile([128, A], BF16, tag="xb")
            nc.vector.tensor_copy(xb, xt[q])
            lhsT = lhsT_pool.tile([66, 128], BF16, tag="lhsT")
            if jk:
                tp = psum_pool.tile([64, 128], BF16, tag="lin")
                nc.tensor.transpose(tp, xb, identb)
                nc.vector.tensor_scalar_mul(out=lhsT[0:64, :], in0=tp,
                                            scalar1=float(w01))
            else:
                for nl in range(2):
                    nc.vector.tensor_scalar_mul(
                        out=lhsT[0:64, 64 * nl:64 * nl + 64],
                        in0=xb[64 * nl:64 * nl + 64, :], scalar1=float(w01))
            nc.sync.dma_start(out=lhsT[64:66, :], in_=indt)
            for nl in range(2):
                nc.sync.dma_start(out=mega[64 + nl:65 + nl, :],
                                  in_=xb[64 * nl:64 * nl + 64, :])

            cml = c_pool.tile([128, A], F32, tag="c")
            for hh in range(2):
                lin = psum_pool.tile([128, NCH * 512], F32, tag="lin")
                for h in range(NCH):
                    if jk:  # free = (j major, k minor); chunk = 8 j x 64 k
                        j0 = HJ * hh + 8 * h
                        rhs = mega[0:66, j0 * A:(j0 + 8) * A]
                    else:  # free = (k major, j minor); chunk = 16 k x 32 j
                        mv = mega[0:66, :].rearrange("p (k j) -> p k j", j=A)
                        rhs = mv[:, 16 * h:16 * h + 16, HJ * hh:HJ * hh + HJ]
                    nc.tensor.matmul(lin[:, 512 * h:512 * (h + 1)], lhsT[0:66, :],
                                     rhs, start=True, stop=True)
                sig = s_pool.tile([128, NCH * 512], BF16, tag="S")
                nc.scalar.activation(sig, lin, AF.Sigmoid, bias=bias_t, scale=1.0)
                t1 = t1_pool.tile([128, 32, 32], BF16, tag="t1")
                if jk:
                    sv = sig.rearrange("p (j k) -> p j k", k=A)
                    nc.gpsimd.tensor_add(out=t1, in0=sv[:, :, 0:32],
                                         in1=sv[:, :, 32:64])
                    tv = t1
                else:
                    sv = sig.rearrange("p (k j) -> p k j", j=HJ)
                    nc.gpsimd.tensor_add(out=t1, in0=sv[:, 0:32, :],
                                         in1=sv[:, 32:64, :])
                    tv = t1.rearrange("p a b -> p b a")
                nc.vector.reduce_sum(out=cml[:, HJ * hh:HJ * (hh + 1)], in_=tv,
                                     axis=AX.X)
            if dbg and name == "a1":
                nc.sync.dma_start(out=dbg["dbg_c1"][q], in_=cml)
            maxT = st_pool.tile([128, 1], F32, tag="maxT")
            sumT = st_pool.tile([128, 1], F32, tag="sumT")
            nc.vector.reduce_max(out=maxT, in_=cml, axis=AX.X)
            nc.vector.reduce_sum(out=sumT, in_=cml, axis=AX.X)
            rin = st_pool.tile([128, 1], F32, tag="rin")
            nc.vector.tensor_scalar_add(out=rin, in0=maxT, scalar1=float(A * EPS))
            rec = st_pool.tile([128, 1], F32, tag="rec")
            nc.vector.reciprocal(rec, rin)
            t2 = st_pool.tile([128, 1], F32, tag="t2")
            nc.vector.tensor_mul(out=t2, in0=sumT, in1=rec)
            nc.vector.tensor_scalar(out=zcol[:, q:q + 1], in0=t2,
                                    scalar1=float(sharp) / A, scalar2=float(th),
                                    op0=ALU.mult, op1=ALU.add)
            nc.vector.tensor_scalar_max(out=zcol[:, q:q + 1], in0=zcol[:, q:q + 1],
                                        scalar1=0.0)
        # softmax over i (= 64-blocks of the partition dim)
        ztp = psum_pool.tile([NPAIR, 128], F32, tag="lin")
        nc.tensor.transpose(ztp, zcol, ident)
        zt = post_pool.tile([NPAIR, 128], F32, tag="zt")
        nc.vector.tensor_copy(zt, ztp)
        vec_t = post_pool.tile([NPAIR, 128], F32, tag="vt")
        for nl in range(2):
            blk = zt[:, 64 * nl:64 * nl + 64]
            mx = st_pool.tile([NPAIR, 1], F32, tag="mx")
            nc.vector.reduce_max(out=mx, in_=blk, axis=AX.X)
            nmx = st_pool.tile([NPAIR, 1], F32, tag="nmx")
            nc.vector.tensor_scalar_mul(out=nmx, in0=mx, scalar1=-1.0)
            u = post_pool.tile([NPAIR, 64], F32, tag="u")
            nc.vector.tensor_scalar_add(out=u, in0=blk, scalar1=nmx)
            sp = post_pool.tile([NPAIR, 64], F32, tag="sp")
            nc.scalar.activation(sp, u, AF.Sigmoid, bias=z8)
            sn = post_pool.tile([NPAIR, 64], F32, tag="sn")
            nc.scalar.activation(sn, u, AF.Sigmoid, bias=z8, scale=-1.0)
            rn = post_pool.tile([NPAIR, 64], F32, tag="rn")
            nc.vector.reciprocal(rn, sn)
            e = post_pool.tile([NPAIR, 64], F32, tag="e")
            nc.vector.tensor_mul(out=e, in0=sp, in1=rn)
            s = st_pool.tile([NPAIR, 1], F32, tag="s")
            nc.vector.reduce_sum(out=s, in_=e, axis=AX.X)
            fr = st_pool.tile([NPAIR, 1], F32, tag="fr")
            if last:
                nc.vector.reciprocal(fr, s)
            else:
                sf = st_pool.tile([NPAIR, 1], F32, tag="sf")
                nc.vector.tensor_scalar(out=sf, in0=s, scalar1=EPS, scalar2=1.0,
                                        op0=ALU.mult, op1=ALU.add)
                nc.vector.reciprocal(fr, sf)
            nc.vector.tensor_scalar_mul(out=vec_t[:, 64 * nl:64 * nl + 64], in0=e,
                                        scalar1=fr)
        if dbg and name == "a1":
            nc.sync.dma_start(out=dbg["dbg_z1"], in_=zcol)
            nc.sync.dma_start(out=dbg["dbg_vec1"], in_=vec_t)
        if dbg and name == "a2":
            nc.sync.dma_start(out=dbg["dbg_vec2"], in_=vec_t)
        return vec_t

    # att1 on h0 (JK layout)
    vec1_t = att(h0, megaJK, True, indT["i1"], p["w01_1"], p["b_1"],
                 p["sharp_1"], p["th_1"], False, "a1")
    vcp = psum_pool.tile([128, NPAIR], F32, tag="lin")
    nc.tensor.transpose(vcp, vec1_t, ident[0:NPAIR, 0:NPAIR])
    vcols = post_pool.tile([128, NPAIR], F32, tag="vcols")
    nc.vector.tensor_copy(vcols, vcp)
    for q in range(NPAIR):
        nc.vector.tensor_scalar_mul(out=h0s[q], in0=h0[q], scalar1=vcols[:, q:q + 1])
        nc.vector.tensor_scalar_mul(out=h1s[q], in0=h1[q], scalar1=vcols[:, q:q + 1])

    # att2 on transpose(h1s) (KJ layout)
    vec2_t = att(h1s, megaKJ, False, indT["i2"], p["w01_2"], p["b_2"],
                 p["sharp_2"], p["th_2"], False, "a2")
    # vnq[nl, pair*64+q] = vec2[2*pair+nl, q]  (base-partition-0 slices)
    vnq = post_pool.tile([2, NPAIR * 64], BF16, tag="vnq")
    for nl in range(2):
        nc.gpsimd.dma_start(out=vnq[nl:nl + 1, :], in_=vec2_t[:, 64 * nl:64 * nl + 64])
    for q in range(NPAIR):
        vb = psum_pool.tile([128, 64], F32, tag="lin")
        nc.tensor.matmul(vb, ind2, vnq[0:2, 64 * q:64 * q + 64], start=True,
                         stop=True)
        nc.vector.tensor_mul(out=x3[q], in0=h0s[q], in1=vb)
    if dbg:
        for q in range(NPAIR):
            nc.sync.dma_start(out=dbg["dbg_x3"][q], in_=x3[q])

    # att3 on x3 (JK layout), is_last
    vec3_t = att(x3, megaJK, True, indT["i3"], p["w01_3"], p["b_3"],
                 p["sharp_3"], p["th_3"], True, "a3")
    out_r = out_ext.rearrange("(q t) a -> q t a", t=2)
    for nl in range(2):
        nc.sync.dma_start(out=out_r[:, nl, :], in_=vec3_t[:, 64 * nl:64 * nl + 64])
    ctx.close()


_CACHE = {}


def _params(fc_W, fc_b, p1_W, p1_b, p1_sharp, p1_th, p2_W, p2_b, p2_sharp, p2_th,
            out_W, out_b, out_sharp, out_th):
    return {
        "fc": [[float(fc_W[0, 0]), float(fc_W[0, 1])],
               [float(fc_W[1, 0]), float(fc_W[1, 1])]],
        "fcb": [float(fc_b[0]), float(fc_b[1])],
        "w00_1": float(p1_W[0, 0]), "w01_1": float(p1_W[0, 1]),
        "b_1": float(p1_b[0]), "sharp_1": float(p1_sharp[0]), "th_1": float(p1_th[0]),
        "w00_2": float(p2_W[0, 0]), "w01_2": float(p2_W[0, 1]),
        "b_2": float(p2_b[0]), "sharp_2": float(p2_sharp[0]), "th_2": float(p2_th[0]),
        "w00_3": float(out_W[0, 0]), "w01_3": float(out_W[0, 1]),
        "b_3": float(out_b[0]), "sharp_3": float(out_sharp[0]),
        "th_3": float(out_th[0]),
    }


def kernel(x, fc_W, fc_b, p1_W, p1_b, p1_sharp, p1_th, p2_W, p2_b, p2_sharp, p2_th,
           out_W, out_b, out_sharp, out_th, _debug=False, _trace=False):
    x = np.ascontiguousarray(np.asarray(x, dtype=np.float32))
    assert x.shape == (N_FULL, 2, A, A), x.shape
    p = _params(fc_W, fc_b, p1_W, p1_b, p1_sharp, p1_th, p2_W, p2_b, p2_sharp,
                p2_th, out_W, out_b, out_sharp, out_th)
    key = (tuple(sorted((k, str(v)) for k, v in p.items())), _debug)
    if key not in _CACHE:
        _CACHE[key] = _build(p, debug=_debug)
    nc = _CACHE[key]
    in_maps = [{"x": x[i * NLOC:(i + 1) * NLOC]} for i in range(N_CORES)]
    r = run_bass_kernel_spmd(nc, in_maps, list(range(N_CORES)), trace=_trace)
    out = np.concatenate([r.results[i]["out"] for i in range(N_CORES)], axis=0)
    if _debug or _trace:
        return out, r
    return out
